# revision 39
# baseline (speedup 1.0000x reference)
"""Trainium2 Bass kernel for gated GQA attention (nn_Attention_6476810683032).

Sharding: 8 cores = 2 (batch DP) x 4 (head-group TP).
Core c handles batch b=c//4, head group g=c%4 (q-heads 4g..4g+3, kv-head g).
Each core computes a partial o_proj output [D, T] (its 4 heads' contribution,
transposed layout); the host sums the 4 partials per batch and transposes.

On-device per core (all matmuls bf16 with fp32 PSUM accumulation):
  - projections from host-pre-transposed hidden_t [D, T] (channel-major
    outputs for q/gate/k, token-major for v) -- no on-device transposes
  - RMS norm via ones-matmul partition reduction + K=1 broadcast matmul
  - RoPE via partition-offset elementwise ops with a pre-signed sin table
  - causal attention in transposed-score form: S_T[tk,tq] = k_rot.T@q_rot,
    exp without max subtraction (logits bounded by the RMS norms),
    denominator via ones-matmul, PV matmul accumulates attn_T[k,tq]
  - sigmoid gating fused with the softmax normalization (2 DVE ops)
  - partial o_proj: out_T[dout,t] = wo_slice.T @ gated (bf16 partials,
    summed in f32 on the host)

Scheduling notes (all engines execute their streams in order, so emission
order is the schedule):
  - phase 0 is emitted d-outer so k/v/first-q projections pace with the
    hidden-load DMA stream (startup is HBM-bandwidth-bound, ~360 GB/s
    across 16 SDMA engines)
  - per chunk: projection pairs with norm/rope chains sandwiched between
    them, then attention with the two head-pairs' m-loops interleaved
    (exp of one head hides behind the scores matmul of the other)
  - o_proj of chunk c-1 (and for chunk 0, the next chunk's first
    projection pair) is drip-fed between attention m-steps as PE filler
  - softmax denominators broadcast to [128,512] via a K=1 ones-matmul and
    inverted with the fast custom-DVE reciprocal off the PE critical path
  - sigmoids are explicitly ordered after the chunk's norm chains on ACT
    (a scheduler dependency) to avoid ACT function-table reload thrash
Measured: ~418 us on-silicon (from 803 us for the first correct version);
PE streaming floor for this decomposition is ~290 us.
"""

import os
import sys
from contextlib import ExitStack

import numpy as np

sys.path.insert(0, "/opt/trn_rl_repo")

import ml_dtypes  # noqa: E402

import concourse.bass as bass  # noqa: E402
import concourse.mybir as mybir  # noqa: E402
import concourse.tile as tile  # noqa: E402
from concourse import bacc  # noqa: E402
from concourse import masks as masks_mod  # noqa: E402

F32 = mybir.dt.float32
BF16 = mybir.dt.bfloat16
AF = mybir.ActivationFunctionType
ALU = mybir.AluOpType
BF = ml_dtypes.bfloat16

P = 128
B, T, D = 2, 2048, 2048
NH, NKV, HD = 16, 4, 128
NHL = NH // NKV          # local q heads per core (4)
CH = 4                   # tq chunks
CT = T // CH             # 512 tokens per chunk
DT = D // P              # 16 contraction tiles
KT = T // P              # 16 tk tiles
EPS = 1e-6
SCALE = HD ** -0.5
N_CORES = 8


def _norm_rope(nc, pools, psr, pss, ones_col, ones_row, eps_t, x_bf, w_ap,
               cos_sl, sin_sl, out_ap, n):
    """RMS-norm (over partitions) + RoPE on a [128, n] channel-major tile.

    x_bf: [128, n] bf16 SBUF (pre-norm channels-on-partitions tile)
    w_ap: [128, 1] f32 norm weight
    cos_sl/sin_sl: [128, n] bf16 (sin pre-signed: rows 0-63 negated)
    out_ap: [128, n] bf16 destination
    """
    sbw, sbr = pools
    xsq = sbw.tile([P, n], BF16, tag="tmpa", name="xsq")
    nc.vector.tensor_tensor(xsq[:], x_bf, x_bf, op=ALU.mult)
    ssq = psr.tile([1, n], F32, tag="row", name="ssq")
    nc.tensor.matmul(ssq[:], ones_col, xsq[:], start=True, stop=True)
    rsq = sbr.tile([1, n], BF16, tag="rsq", name="rsq")
    absr = nc.scalar.activation(rsq[:], ssq[:], AF.Abs_reciprocal_sqrt,
                                scale=1.0 / HD, bias=eps_t)
    rb = pss.tile([P, n], F32, tag="ss", name="rb")
    nc.tensor.matmul(rb[:], ones_row, rsq[:], start=True, stop=True)
    wr = sbw.tile([P, n], BF16, tag="tmpb", name="wr")
    nc.vector.tensor_scalar(wr[:], rb[:], w_ap, None, ALU.mult)
    xn = sbw.tile([P, n], BF16, tag="xn", name="xn")
    nc.vector.tensor_tensor(xn[:], x_bf, wr[:], op=ALU.mult)
    t1 = sbw.tile([P, n], BF16, tag="tmpb", name="t1")
    nc.vector.tensor_tensor(t1[:], xn[:], cos_sl, op=ALU.mult)
    h = HD // 2
    xs = sbw.tile([P, n], BF16, tag="tmpc", name="xs")
    nc.vector.tensor_copy(xs[0:h, :], xn[h:P, :])
    nc.vector.tensor_copy(xs[h:P, :], xn[0:h, :])
    t2 = sbw.tile([P, n], BF16, tag="tmpa", name="t2")
    nc.vector.tensor_tensor(t2[:], xs[:], sin_sl, op=ALU.mult)
    nc.vector.tensor_tensor(out_ap, t1[:], t2[:], op=ALU.add)
    return absr


def build_nc():
    nc = bacc.Bacc("TRN2", target_bir_lowering=False, debug=False,
                   num_devices=N_CORES)
    hid_d = nc.dram_tensor("hid", [D, T], BF16, kind="ExternalInput")
    wqq_d = nc.dram_tensor("wqq", [D, NHL * HD], BF16, kind="ExternalInput")
    wqg_d = nc.dram_tensor("wqg", [D, NHL * HD], BF16, kind="ExternalInput")
    wk_d = nc.dram_tensor("wk", [D, HD], BF16, kind="ExternalInput")
    wv_d = nc.dram_tensor("wv", [D, HD], BF16, kind="ExternalInput")
    wo_d = nc.dram_tensor("wo", [NHL * HD, D], BF16, kind="ExternalInput")
    cos_d = nc.dram_tensor("cost", [P, T], BF16, kind="ExternalInput")
    sin_d = nc.dram_tensor("sinpm", [P, T], BF16, kind="ExternalInput")
    qw_d = nc.dram_tensor("qw", [P, 1], F32, kind="ExternalInput")
    kw_d = nc.dram_tensor("kw", [P, 1], F32, kind="ExternalInput")
    mask_d = nc.dram_tensor("masks", [P, NHL * CT], BF16, kind="ExternalInput")
    out_d = nc.dram_tensor("out_t", [D, T], BF16, kind="ExternalOutput")

    with tile.TileContext(nc) as tc, ExitStack() as ctx, \
            nc.allow_low_precision(reason="bf16 softmax temps validated by rel_err"):
        sbp = ctx.enter_context(tc.tile_pool(name="sbp", bufs=1))
        sbw = ctx.enter_context(tc.tile_pool(name="sbw", bufs=3))
        sbr = ctx.enter_context(tc.tile_pool(name="sbr", bufs=2))
        sbq = ctx.enter_context(tc.tile_pool(name="sbq", bufs=6))
        psp = ctx.enter_context(tc.tile_pool(name="psp", bufs=2, space="PSUM"))
        pss = ctx.enter_context(tc.tile_pool(name="pss", bufs=2, space="PSUM"))
        psa = ctx.enter_context(tc.tile_pool(name="psa", bufs=2, space="PSUM"))
        psr = ctx.enter_context(tc.tile_pool(name="psr", bufs=2, space="PSUM"))

        # ---- persistent tiles + loads ----
        # Loads ordered by first use (k/v proj -> chains -> q proj -> o_proj)
        # and spread across DMA queues of otherwise-idle engines.
        wk = []
        for i in range(DT):
            t = sbp.tile([P, HD], BF16, tag=f"wk{i}", name=f"wk{i}")
            nc.sync.dma_start(t[:], wk_d[i * P:(i + 1) * P, :])
            wk.append(t)
        hid = []
        for i in range(DT):
            t = sbp.tile([P, T], BF16, tag=f"hid{i}", name=f"hid{i}")
            q = (nc.sync, nc.scalar, nc.gpsimd)[i % 3]
            q.dma_start(t[:], hid_d[i * P:(i + 1) * P, :])
            hid.append(t)
        wv = []
        for i in range(DT):
            t = sbp.tile([P, HD], BF16, tag=f"wv{i}", name=f"wv{i}")
            nc.scalar.dma_start(t[:], wv_d[i * P:(i + 1) * P, :])
            wv.append(t)
        cost = sbp.tile([P, T], BF16, tag="cost")
        nc.gpsimd.dma_start(cost[:], cos_d[:, :])
        sinpm = sbp.tile([P, T], BF16, tag="sinpm")
        nc.gpsimd.dma_start(sinpm[:], sin_d[:, :])
        masks = sbp.tile([P, NHL * CT], BF16, tag="masks")
        nc.gpsimd.dma_start(masks[:], mask_d[:, :])
        qw = sbp.tile([P, 1], F32, tag="qw")
        nc.gpsimd.dma_start(qw[:], qw_d[:, :])
        kw = sbp.tile([P, 1], F32, tag="kw")
        nc.gpsimd.dma_start(kw[:], kw_d[:, :])
        wqq = []
        wqg = []
        for i in range(DT):
            t = sbp.tile([P, NHL * HD], BF16, tag=f"wqq{i}", name=f"wqq{i}")
            nc.sync.dma_start(t[:], wqq_d[i * P:(i + 1) * P, :])
            wqq.append(t)
            t = sbp.tile([P, NHL * HD], BF16, tag=f"wqg{i}", name=f"wqg{i}")
            nc.sync.dma_start(t[:], wqg_d[i * P:(i + 1) * P, :])
            wqg.append(t)
        wo = []
        for i in range(NHL):
            t = sbp.tile([P, D], BF16, tag=f"wo{i}", name=f"wo{i}")
            nc.sync.dma_start(t[:], wo_d[i * P:(i + 1) * P, :])
            wo.append(t)
        ones_col = sbp.tile([P, 1], BF16, tag="ones_col")
        nc.vector.memset(ones_col[:], 1.0)
        ones_row = sbp.tile([1, P], BF16, tag="ones_row")
        nc.vector.memset(ones_row[:], 1.0)
        eps_t = sbp.tile([1, 1], F32, tag="eps_t")
        nc.vector.memset(eps_t[:], EPS)
        ident = sbp.tile([P, P], BF16, tag="ident")
        masks_mod.make_identity(nc, ident[:])
        krot = sbp.tile([P, T], BF16, tag="krot")
        vsb = []
        for i in range(KT):
            vsb.append(sbp.tile([P, HD], BF16, tag=f"v{i}", name=f"v{i}"))

        # ---- phase 0: everything d-outer so the PE paces with the hid DMA
        # stream (each matmul needs only hid[d]). Per d: 4 k-proj, 2 v-proj
        # (chunks 0-1), and chunk-0's first q/gate projection pair.
        kps = [pss.tile([P, CT], F32, tag="ss", name="kps0"),
               pss.tile([P, CT], F32, tag="ss", name="kps1"),
               psa.tile([P, CT], F32, tag="aa", name="kps2"),
               psa.tile([P, CT], F32, tag="aa", name="kps3")]
        vps01 = [psr.tile([P, CT], F32, tag="row", name="vps0"),
                 psr.tile([P, CT], F32, tag="row", name="vps1")]
        qp0 = psp.tile([P, CT], F32, tag="pp", name="qp0")
        gp0 = psp.tile([P, CT], F32, tag="pp", name="gp0")
        cs0 = slice(0, CT)
        for d in range(DT):
            st, sp = (d == 0), (d == DT - 1)
            for c in range(CH):
                cs = slice(c * CT, (c + 1) * CT)
                nc.tensor.matmul(kps[c][:], wk[d][:, :], hid[d][:, cs],
                                 start=st, stop=sp)
            for c in range(2):
                cs = slice(c * CT, (c + 1) * CT)
                nc.tensor.matmul(vps01[c][:], wv[d][:, :], hid[d][:, cs],
                                 start=st, stop=sp)
            nc.tensor.matmul(qp0[:], wqq[d][:, 0:HD], hid[d][:, cs0],
                             start=st, stop=sp)
            nc.tensor.matmul(gp0[:], wqg[d][:, 0:HD], hid[d][:, cs0],
                             start=st, stop=sp)
        kbfs = []
        for c in range(CH):
            kbf = sbw.tile([P, CT], BF16, tag="kbf", name="kbf", bufs=4)
            nc.vector.tensor_copy(kbf[:], kps[c][:])
            kbfs.append(kbf)
        vct = sbp.tile([P, T], BF16, tag="vct")
        for c in range(2):
            cs = slice(c * CT, (c + 1) * CT)
            nc.vector.tensor_copy(vct[:, cs], vps01[c][:])
        q_sb0 = sbq.tile([P, CT], BF16, tag="q_sb", bufs=4, name="q_sb0")
        nc.vector.tensor_copy(q_sb0[:], qp0[:])
        g_sb0 = sbq.tile([P, CT], BF16, tag="g_sb", bufs=5, name="g_sb0")
        nc.vector.tensor_copy(g_sb0[:], gp0[:])
        pre_pairs = {0: (q_sb0, g_sb0)}
        for c in range(2, CH):
            cs = slice(c * CT, (c + 1) * CT)
            ps = psr.tile([P, CT], F32, tag="row", name="vcps")
            for d in range(DT):
                nc.tensor.matmul(ps[:], wv[d][:, :], hid[d][:, cs],
                                 start=(d == 0), stop=(d == DT - 1))
            nc.vector.tensor_copy(vct[:, cs], ps[:])
        for tt in range(KT):
            tps = pss.tile([P, P], BF16, tag="ss", name="tps")
            nc.tensor.transpose(tps[:], vct[:, tt * P:(tt + 1) * P],
                                ident[:])
            nc.vector.tensor_copy(vsb[tt][:], tps[:])

        # ---- phase 1: per tq-chunk: q/gate proj, attention ----
        # o_proj for chunk c-1 is emitted after chunk c's norm chains so the
        # PE has dense work while the chains' DVE/ACT latency drains.
        def _o_proj(oc, og):
            ocs = slice(oc * CT, (oc + 1) * CT)
            # the final o_proj has the PSUM to itself: rotate all four tag
            # groups so psum-free never gates the accumulation chains
            pools4 = [(psp, "pp"), (pss, "ss"), (psa, "aa"), (psr, "row")]
            for dt in range(DT):
                ds_ = slice(dt * P, (dt + 1) * P)
                if oc == CH - 1:
                    pl, tg = pools4[dt % 4]
                    pso = pl.tile([P, CT], F32, tag=tg, name="pso")
                else:
                    pso = psp.tile([P, CT], F32, tag="pp", name="pso")
                for ct4 in range(NHL):
                    nc.tensor.matmul(pso[:], wo[ct4][:, ds_], og[ct4][:],
                                     start=(ct4 == 0), stop=(ct4 == NHL - 1))
                osb = sbw.tile([P, CT], BF16, tag="osb", bufs=2, name="osb")
                nc.vector.tensor_copy(osb[:], pso[:])
                nc.sync.dma_start(out_d[ds_, ocs], osb[:])

        prev_gated = None
        for c in range(CH):
            cs = slice(c * CT, (c + 1) * CT)
            q_sbs = []
            g_sbs = []
            sigs = []
            qrots = []

            chain_absr = []

            def _q_chain(h, c=None, cs=None, q_sbs=None, qrots=None):
                qrot = sbw.tile([P, CT], BF16, tag="qrot", bufs=5,
                                name="qrot")
                a = _norm_rope(nc, (sbw, sbr), psr, pss, ones_col[:],
                               ones_row[:], eps_t[:], q_sbs[h][:], qw[:],
                               cost[:, cs], sinpm[:, cs], qrot[:], CT)
                qrots.append(qrot)
                chain_absr.append(a)

            for h in range(NHL):
                if h == 0 and c in pre_pairs:
                    q_sbs.append(pre_pairs[c][0])
                    g_sbs.append(pre_pairs[c][1])
                    chain_absr.append(_norm_rope(
                        nc, (sbw, sbr), psr, pss, ones_col[:],
                        ones_row[:], eps_t[:], kbfs[c][:], kw[:],
                        cost[:, cs], sinpm[:, cs], krot[:, cs], CT))
                    continue
                hs = slice(h * HD, (h + 1) * HD)
                ps = psp.tile([P, CT], F32, tag="pp")
                for d in range(DT):
                    nc.tensor.matmul(ps[:], wqq[d][:, hs], hid[d][:, cs],
                                     start=(d == 0), stop=(d == DT - 1))
                q_sb = sbq.tile([P, CT], BF16, tag="q_sb", bufs=4)
                nc.vector.tensor_copy(q_sb[:], ps[:])
                q_sbs.append(q_sb)
                ps2 = psp.tile([P, CT], F32, tag="pp")
                for d in range(DT):
                    nc.tensor.matmul(ps2[:], wqg[d][:, hs], hid[d][:, cs],
                                     start=(d == 0), stop=(d == DT - 1))
                g_sb = sbq.tile([P, CT], BF16, tag="g_sb", bufs=5)
                nc.vector.tensor_copy(g_sb[:], ps2[:])
                g_sbs.append(g_sb)
                # sandwich a norm/rope chain after each proj pair so the
                # chain's DVE/ACT latency hides behind the next pair's mms
                if h == 0:
                    chain_absr.append(_norm_rope(
                        nc, (sbw, sbr), psr, pss, ones_col[:],
                        ones_row[:], eps_t[:], kbfs[c][:], kw[:],
                        cost[:, cs], sinpm[:, cs], krot[:, cs], CT))
                else:
                    _q_chain(h - 1, c=c, cs=cs, q_sbs=q_sbs, qrots=qrots)
            _q_chain(NHL - 1, c=c, cs=cs, q_sbs=q_sbs, qrots=qrots)
            for h in range(NHL):
                sig = sbq.tile([P, CT], BF16, tag="sig", bufs=5, name="sig")
                si = nc.scalar.activation(sig[:], g_sbs[h][:], AF.Sigmoid)
                # order sigmoids after the chunk's norm chains on ACT so the
                # scheduler doesn't interleave them (each switch reloads the
                # ACT function table, ~1.3us)
                bass._add_dep_helper(si.ins, chain_absr[-1].ins, sync=False,
                                     reason="group sigmoids after absrsqrt")
                sigs.append(sig)
            gated = []
            nm = 4 * c + 4
            # Filler work drip-fed between attention m-steps keeps the PE
            # dense while ACT runs the exps: o_proj(c-1) tiles, and for
            # chunk 0 (which has no prior o_proj) the next chunk's first
            # projection pair.
            fillers = []
            if prev_gated is not None:
                ocs = slice((c - 1) * CT, c * CT)

                def _mk_oproj(dt, ocs=ocs, og=prev_gated):
                    def run():
                        ds_ = slice(dt * P, (dt + 1) * P)
                        pso = psp.tile([P, CT], F32, tag="pp", name="pso")
                        for ct4 in range(NHL):
                            nc.tensor.matmul(pso[:], wo[ct4][:, ds_],
                                             og[ct4][:], start=(ct4 == 0),
                                             stop=(ct4 == NHL - 1))
                        osb = sbw.tile([P, CT], BF16, tag="osb", bufs=2,
                                       name="osb")
                        nc.vector.tensor_copy(osb[:], pso[:])
                        nc.sync.dma_start(out_d[ds_, ocs], osb[:])
                    return run
                fillers += [_mk_oproj(dt) for dt in range(DT)]
            if c == 0:
                cs1 = slice(CT, 2 * CT)
                qp1 = psp.tile([P, CT], F32, tag="pp", name="qp1")
                gp1 = psp.tile([P, CT], F32, tag="pp", name="gp1")

                def _mk_proj(ps_t, w_t, dlist):
                    def run():
                        for d in dlist:
                            nc.tensor.matmul(
                                ps_t[:], w_t[d][:, 0:HD], hid[d][:, cs1],
                                start=(d == 0), stop=(d == DT - 1))
                    return run
                for d0 in range(0, DT, 4):
                    fillers.append(_mk_proj(qp1, wqq, range(d0, d0 + 4)))
                for d0 in range(0, DT, 4):
                    fillers.append(_mk_proj(gp1, wqg, range(d0, d0 + 4)))
            fill = {"i": 0}
            n_steps = 2 * nm

            def _fill_tick(step):
                due = min(len(fillers),
                          len(fillers) * (step + 1) // n_steps + 1)
                while fill["i"] < due:
                    fillers[fill["i"]]()
                    fill["i"] += 1

            step_no = [0]
            for hp in (0, 2):
                pair = (hp, hp + 1)
                denoms = {h: psr.tile([1, CT], F32, tag="row",
                                      name=f"denom{h}") for h in pair}
                attns = {h: psa.tile([P, CT], F32, tag="aa",
                                     name=f"attn{h}") for h in pair}
                for m in range(nm):
                    ks = slice(m * P, (m + 1) * P)
                    r = m - 4 * c
                    lo = P * r if r > 0 else 0
                    ns = slice(lo, CT)
                    for h in pair:
                        sps = pss.tile([P, CT], F32, tag="ss", name="sps")
                        nc.tensor.matmul(sps[:, ns], krot[:, ks],
                                         qrots[h][:, ns],
                                         start=True, stop=True)
                        E = sbw.tile([P, CT], BF16, tag="E", name="E",
                                     bufs=6)
                        nc.scalar.activation(E[:, ns], sps[:, ns], AF.Exp,
                                             scale=SCALE)
                        if r >= 0:
                            nc.gpsimd.tensor_tensor(
                                E[:, lo:lo + P], E[:, lo:lo + P],
                                masks[:, 0:P], op=ALU.mult)
                        nc.tensor.matmul(denoms[h][:, ns], ones_col[:],
                                         E[:, ns], start=(m == 0),
                                         stop=(m == nm - 1))
                        nc.tensor.matmul(attns[h][:, ns], vsb[m][:],
                                         E[:, ns], start=(m == 0),
                                         stop=(m == nm - 1))
                    _fill_tick(step_no[0])
                    step_no[0] += 1
                for h in pair:
                    dn = sbr.tile([1, CT], BF16, tag="rowtmp", name="dn")
                    nc.vector.tensor_copy(dn[:], denoms[h][:])
                    rb = psr.tile([P, CT], F32, tag="row", name="rb")
                    nc.tensor.matmul(rb[:], ones_row[:], dn[:],
                                     start=True, stop=True)
                    rcb = sbw.tile([P, CT], F32, tag="rcb", name="rcb",
                                   bufs=3)
                    nc.vector.reciprocal_approx_fast(out=rcb[:], in_=rb[:])
                    tmp = sbw.tile([P, CT], BF16, tag="tmpc", name="tmp")
                    nc.vector.tensor_tensor(tmp[:], attns[h][:], sigs[h][:],
                                            op=ALU.mult)
                    g = sbq.tile([P, CT], BF16, tag="gated", bufs=9)
                    nc.vector.tensor_tensor(g[:], tmp[:], rcb[:],
                                            op=ALU.mult)
                    gated.append(g)

            while fill["i"] < len(fillers):
                fillers[fill["i"]]()
                fill["i"] += 1
            if c == 0:
                q_sb1 = sbq.tile([P, CT], BF16, tag="q_sb", bufs=4,
                                 name="q_sb1")
                nc.vector.tensor_copy(q_sb1[:], qp1[:])
                g_sb1 = sbq.tile([P, CT], BF16, tag="g_sb", bufs=5,
                                 name="g_sb1")
                nc.vector.tensor_copy(g_sb1[:], gp1[:])
                pre_pairs[1] = (q_sb1, g_sb1)
            prev_gated = gated
        _o_proj(CH - 1, prev_gated)
    nc.compile()
    return nc


def make_in_maps(hidden, cos, sin, wq, wk, wv, wo, q_norm_w, k_norm_w):
    """Build the 8 per-core input maps (host-side sharding + layout prep)."""
    i_idx = np.arange(P)[:, None]
    j_idx = np.arange(CT)[None, :]
    masks = np.concatenate(
        [(j_idx >= i_idx + P * r).astype(BF) for r in range(NHL)], axis=1)
    in_maps = []
    for core in range(N_CORES):
        b, g = core // NKV, core % NKV
        heads = range(NHL * g, NHL * g + NHL)
        sin_t = sin[b].T.copy()
        sin_t[:HD // 2] = -sin_t[:HD // 2]
        in_maps.append({
            "hid": np.ascontiguousarray(hidden[b].T).astype(BF),
            "wqq": np.concatenate(
                [wq[:, h * 2 * HD: h * 2 * HD + HD] for h in heads], 1
            ).astype(BF),
            "wqg": np.concatenate(
                [wq[:, h * 2 * HD + HD: (h + 1) * 2 * HD] for h in heads], 1
            ).astype(BF),
            "wk": np.ascontiguousarray(wk[:, g * HD:(g + 1) * HD]).astype(BF),
            "wv": np.ascontiguousarray(wv[:, g * HD:(g + 1) * HD]).astype(BF),
            "wo": np.ascontiguousarray(
                wo[NHL * HD * g: NHL * HD * (g + 1), :]).astype(BF),
            "cost": np.ascontiguousarray(cos[b].T).astype(BF),
            "sinpm": np.ascontiguousarray(sin_t).astype(BF),
            "qw": np.ascontiguousarray(q_norm_w[:, None]).astype(np.float32),
            "kw": np.ascontiguousarray(k_norm_w[:, None]).astype(np.float32),
            "masks": np.ascontiguousarray(masks),
        })
    return in_maps


def _install_ntff_hook():
    """Inject antenv.axon_hooks with a ctypes NTFF profile hook.

    The container's antenv package lacks axon_hooks, so bass_utils'
    trace=True path can't find the hook. Replicates the boot script's
    _ntff_profile_via_ctypes against libaxon_pjrt.so.
    """
    import contextlib
    import ctypes
    import types

    if "antenv.axon_hooks" in sys.modules:
        return
    lib = None
    for so_path in ("/opt/axon/libaxon_pjrt.so",
                    "/root/.axon_site/axon/libaxon_pjrt.so"):
        try:
            lib = ctypes.CDLL(so_path)
            break
        except OSError:
            continue
    if lib is None:
        return
    if not hasattr(lib, "axon_start_nrt_profile"):
        return
    lib.axon_start_nrt_profile.argtypes = [ctypes.POINTER(ctypes.c_int64),
                                           ctypes.c_size_t]
    lib.axon_start_nrt_profile.restype = ctypes.c_int64
    lib.axon_stop_nrt_profile.argtypes = [ctypes.c_char_p]
    lib.axon_stop_nrt_profile.restype = ctypes.c_int64

    @contextlib.contextmanager
    def _hook(output_dir, device_ids):
        import jax

        jax.devices()
        if device_ids:
            ids = (ctypes.c_int64 * len(device_ids))(*device_ids)
            rc = lib.axon_start_nrt_profile(ids, len(device_ids))
        else:
            rc = lib.axon_start_nrt_profile(None, 0)
        if rc != 0:
            raise RuntimeError(f"axon_start_nrt_profile rc={rc}")
        try:
            yield
        finally:
            n = lib.axon_stop_nrt_profile(str(output_dir).encode())
            print(f"profile: {n} file(s) written to {output_dir}",
                  file=sys.stderr)

    m = types.ModuleType("antenv.axon_hooks")
    m.get_axon_ntff_profile_hook = lambda: _hook
    m.set_axon_ntff_profile_hook = lambda h: None
    sys.modules["antenv.axon_hooks"] = m


_NC_CACHE = None


def _get_nc():
    global _NC_CACHE
    if _NC_CACHE is None:
        _NC_CACHE = build_nc()
    return _NC_CACHE


def kernel(hidden_BTD, cos_BTK, sin_BTK, wq, wk, wv, wo, q_norm_w, k_norm_w,
           segment_ids_BT=None, position_ids_BT=None, **_unused):
    from concourse.bass_utils import run_bass_kernel_spmd

    in_maps = make_in_maps(
        np.asarray(hidden_BTD, np.float32), np.asarray(cos_BTK, np.float32),
        np.asarray(sin_BTK, np.float32), np.asarray(wq, np.float32),
        np.asarray(wk, np.float32), np.asarray(wv, np.float32),
        np.asarray(wo, np.float32), np.asarray(q_norm_w, np.float32),
        np.asarray(k_norm_w, np.float32))
    nc = _get_nc()
    trace = bool(int(os.environ.get("BASS_KERNEL_TRACE", "0")))
    if trace:
        _install_ntff_hook()
    res = run_bass_kernel_spmd(nc, in_maps, core_ids=list(range(N_CORES)),
                               trace=trace)
    out = np.zeros((B, T, D), np.float32)
    for core in range(N_CORES):
        out[core // NKV] += res.results[core]["out_t"].astype(np.float32).T
    kernel.last_exec_time_ns = res.exec_time_ns
    kernel.last_results = res
    return out


kernel.last_exec_time_ns = None
kernel.last_results = None
